# revision 12
# baseline (speedup 1.0000x reference)
"""Trainium2 Bass kernel for out = x * exclusive_cumsum(x, axis=time).

Input x: [B=8, T=4096, D=1024] f32. Pure data parallel: batch element b -> core b.

Per-core algorithm (x_c: [T, D], partition axis = time):
  - T is split into 32 blocks of 128 rows. All 32 x-tiles stay resident in SBUF.
  - Phase A: per block b, a colsum matmul with selector weights (ones only in
    lhsT column b) accumulates the block's column totals into row b of one
    PSUM tile [32, 512] per 512-wide D chunk; one DVE copy moves it to SBUF.
  - Phase B: per block, a strict-upper-triangular 128x128 matmul computes the
    within-block exclusive cumsum into PSUM (start=True); a second matmul with
    step weights wstep[:, 128b:128b+128] (row k all-ones iff k < b) over the
    totals tile adds carry_b = sum_{k<b} totals_k to every partition
    (start=False). DVE multiplies x by the PSUM prefix; result DMAs out.

All bulk DMA traffic is fully-linear 512KB blocks; the kernel is
HBM-bandwidth-bound. Matmuls can run in float32 (exact, 4 cyc/row) or
float32r (1 cyc/row, relaxed precision) via use_f32r.
"""

import sys

sys.path.insert(0, "/opt/trn_rl_repo")

import numpy as np

B, T, D = 8, 4096, 1024
BLK = 128
NBLK = T // BLK  # 32
NCH = 2
CH = D // NCH  # 512, exactly one PSUM bank in f32

_CACHE = {}


def _weights(nblk=NBLK):
    wtri = np.triu(np.ones((BLK, BLK), dtype=np.float32), 1)  # [k,m]=1 iff k<m
    # Selector: ones in column 64 only. lhsT = wsel[:, 64-b : 64-b+nblk] has
    # ones exactly in slice-column b.
    wsel = np.zeros((BLK, BLK), dtype=np.float32)
    wsel[:, 64] = 1.0
    # Step: wstep[k, c] = 1 iff c >= 128*(k+1). Slice [:, 128b : 128b+128]
    # gives row k = all-ones iff k < b, all-zeros otherwise.
    c = np.arange(128 * (nblk + 1))[None, :]
    k = np.arange(nblk)[:, None]
    wstep = (c >= 128 * (k + 1)).astype(np.float32)
    return wtri, wsel, wstep


def build_nc(t=T, d=D, blk=BLK, nch=NCH, use_f32r=True, num_devices=B):
    """Build the Bass module for one core's [t, d] shard."""
    import concourse.bass as bass
    import concourse.mybir as mybir
    import concourse.tile as tile
    from concourse import bacc

    f32 = mybir.dt.float32
    f32r = mybir.dt.float32r
    nblk = t // blk
    ch = d // nch
    assert t % blk == 0 and d % nch == 0 and ch <= 512 and nblk <= 32

    def mm(ap):
        return ap.bitcast(f32r) if use_f32r else ap

    nc = bacc.Bacc("TRN2", target_bir_lowering=False, debug=False,
                   num_devices=num_devices)
    x = nc.dram_tensor("x", [t, d], f32, kind="ExternalInput").ap()
    wtri = nc.dram_tensor("wtri", [blk, blk], f32, kind="ExternalInput").ap()
    wsel = nc.dram_tensor("wsel", [blk, blk], f32, kind="ExternalInput").ap()
    wstep = nc.dram_tensor("wstep", [nblk, 128 * (nblk + 1)], f32,
                           kind="ExternalInput").ap()
    out = nc.dram_tensor("out", [t, d], f32, kind="ExternalOutput").ap()

    with tile.TileContext(nc) as tc:
        with (
            tc.tile_pool(name="wpool", bufs=1) as wpool,
            tc.tile_pool(name="xpool", bufs=1) as xpool,
            tc.tile_pool(name="spool", bufs=1) as spool,
            tc.tile_pool(name="opool", bufs=4) as opool,
        ):
            wdt = f32r if use_f32r else f32

            def wload(shape, tag, src):
                t_ = wpool.tile(shape, wdt, tag=tag)
                nc.sync.dma_start(t_[:], src.bitcast(wdt))
                return t_.bitcast(f32)

            wt = wload([blk, blk], "wt", wtri[:])
            ws = wload([blk, blk], "ws", wsel[:])
            wp = wload([nblk, 128 * (nblk + 1)], "wp", wstep[:])

            xt = []
            for b in range(nblk):
                if use_f32r:
                    t_ = xpool.tile([blk, d], f32r, tag=f"x{b}")
                    nc.sync.dma_start(t_[:],
                                      x[b * blk:(b + 1) * blk, :].bitcast(f32r))
                    t_ = t_.bitcast(f32)
                else:
                    t_ = xpool.tile([blk, d], f32, tag=f"x{b}")
                    nc.sync.dma_start(t_[:], x[b * blk:(b + 1) * blk, :])
                xt.append(t_)

            totals = []
            with tc.tile_pool(name="ptot", bufs=1,
                              space=bass.MemorySpace.PSUM) as ptot:
                for j in range(nch):
                    jc = slice(j * ch, (j + 1) * ch)
                    tot_psum = ptot.tile([nblk, ch], f32, tag=f"tot{j}")
                    for b in range(nblk):
                        nc.tensor.matmul(
                            tot_psum[:],
                            mm(ws[:, 64 - b:64 - b + nblk]),  # col b only
                            mm(xt[b][:, jc]),                 # K=blk, N=ch
                            start=(b == 0), stop=(b == nblk - 1),
                        )
                    tots = spool.tile([nblk, ch], wdt, tag=f"tots{j}")
                    nc.vector.tensor_copy(tots[:], tot_psum[:])
                    totals.append(tots.bitcast(f32))

            with tc.tile_pool(name="pblk", bufs=3,
                              space=bass.MemorySpace.PSUM) as pblk:
                for b in range(nblk):
                    ot = opool.tile([blk, d], f32, tag="out")
                    for j in range(nch):
                        jc = slice(j * ch, (j + 1) * ch)
                        ps = pblk.tile([blk, ch], f32, tag=f"pb{j}")
                        nc.tensor.matmul(
                            ps[:], mm(wt[:]), mm(xt[b][:, jc]),
                            start=True, stop=False,
                        )
                        nc.tensor.matmul(
                            ps[:],
                            mm(wp[:, 128 * b:128 * b + 128]),  # rows k<b ones
                            mm(totals[j][:]),                  # K=nblk, N=ch
                            start=False, stop=True,
                        )
                        nc.any.tensor_mul(ot[:, jc], xt[b][:, jc], ps[:])
                    nc.sync.dma_start(out[b * blk:(b + 1) * blk, :], ot[:])

    nc.compile()
    return nc


def kernel(x: np.ndarray) -> np.ndarray:
    from concourse.bass_utils import run_bass_kernel_spmd

    assert x.shape == (B, T, D) and x.dtype == np.float32
    key = "full"
    if key not in _CACHE:
        _CACHE[key] = build_nc()
    nc = _CACHE[key]

    wtri, wsel, wstep = _weights()
    in_maps = [
        {"x": np.ascontiguousarray(x[c]), "wtri": wtri, "wsel": wsel,
         "wstep": wstep}
        for c in range(B)
    ]
    res = run_bass_kernel_spmd(nc, in_maps, core_ids=list(range(B)))
    return np.stack([res.results[c]["out"] for c in range(B)], axis=0)
